# revision 3
# baseline (speedup 1.0000x reference)
"""Radius neighbor search (dense CSR encoding) on 8 TRN2 NeuronCores.

For M=12288 queries vs N=12288 data points in R^3 and radius r:
  d2[m,n]    = ||q_m||^2 + ||d_n||^2 - 2 q_m.d_n   (fp32-grade)
  mask[m,n]  = d2 <= r^2
  weights    = where(mask, max(d2,0), 0)           (fp32)
  row_splits = [0, cumsum(min(row_count, 255))]    (int32; the oracle's
               jnp.sum(mask, dtype=int32) saturates at 255 on this backend)

Sharding: queries row-parallel across 8 cores (1536 rows each), data
replicated; one SPMD Bass program.

Device computes only w = (d2 <= r2) * (d2 + eps) in bf16 (eps=1e-8 makes
every in-radius weight nonzero, so mask/counts derive from w != 0 on the
host). Per [128 x 2048] chunk:
  PE   : 4 matmuls, K=33 bf16 (3-way split of fp32 -> fp32-grade d2)
  ACT  : d2e = d2 + eps, PSUM -> SBUF f32
  DVE  : w = (d2_psum <= r2) * d2e   (scalar_tensor_tensor, ~60% of chunks)
  POOL : maskf = (d2e <= r2+eps); w = maskf * d2e  (~40% of chunks)
  DMA  : w chunk -> DRAM
"""
import numpy as np
import ml_dtypes

import concourse.bacc as bacc
import concourse.mybir as mybir
import concourse.tile as tile
from concourse.bass_utils import run_bass_kernel_spmd

BF16 = ml_dtypes.bfloat16

M = 12288
N = 12288
DIM = 3
NCORES = 8
MPC = M // NCORES   # 1536
K = 33

CHUNK = 2048
NCHUNK = N // CHUNK  # 6
BLOCKS = MPC // 128  # 12
SUB = CHUNK // 512   # 4
EPS = 1e-8
# cycle of 5 chunk-slots: last two go to POOL (~29/72 chunks)
POOL_PATTERN = [False, False, False, True, True]

LAST_RESULT = None
_nc_cache = {}


def _split3(x):
    x = np.asarray(x, np.float32)
    a = x.astype(BF16)
    r = x - a.astype(np.float32)
    b = r.astype(BF16)
    r2 = r - b.astype(np.float32)
    c = r2.astype(BF16)
    return a, b, c


def _build_aug(queries, data):
    """lhsT [K, M] bf16, rhs [K, N] bf16 with lhsT.T@rhs ~= d2 at fp32 grade."""
    q = np.asarray(queries, np.float32)
    d = np.asarray(data, np.float32)
    m, n = q.shape[0], d.shape[0]
    sq_q = np.sum(q * q, axis=1, dtype=np.float32)
    sq_d = np.sum(d * d, axis=1, dtype=np.float32)
    g = (-2.0 * d).astype(np.float32)

    qs = _split3(q)
    gs = _split3(g)
    ss = _split3(sq_q)
    ts_ = _split3(sq_d)

    ones_m = np.ones((m,), BF16)
    ones_n = np.ones((n,), BF16)
    lhs_rows, rhs_rows = [], []

    def add(lr, rr):
        lhs_rows.append(lr.astype(BF16))
        rhs_rows.append(rr.astype(BF16))

    add(ss[0], ones_n)
    add(ones_m, ts_[0])
    for k in range(DIM):
        add(qs[0][:, k], gs[0][:, k])
    add(ss[1], ones_n)
    add(ones_m, ts_[1])
    for (i, j) in [(0, 1), (1, 0)]:
        for k in range(DIM):
            add(qs[i][:, k], gs[j][:, k])
    add(ss[2], ones_n)
    add(ones_m, ts_[2])
    for (i, j) in [(1, 1), (0, 2), (2, 0), (1, 2), (2, 1), (2, 2)]:
        for k in range(DIM):
            add(qs[i][:, k], gs[j][:, k])

    lhsT = np.ascontiguousarray(np.stack(lhs_rows, axis=0))
    rhs = np.ascontiguousarray(np.stack(rhs_rows, axis=0))
    assert lhsT.shape == (K, m) and rhs.shape == (K, n)
    return lhsT, rhs


def _build_nc(r2: float):
    r2e = float(np.float32(r2) + np.float32(EPS))
    nc = bacc.Bacc("TRN2", target_bir_lowering=False, debug=False)
    qaugT = nc.dram_tensor("qaugT", [K, MPC], mybir.dt.bfloat16, kind="ExternalInput")
    daugT = nc.dram_tensor("daugT", [K, N], mybir.dt.bfloat16, kind="ExternalInput")
    w_out = nc.dram_tensor("w_out", [MPC, N], mybir.dt.bfloat16, kind="ExternalOutput")

    with tile.TileContext(nc) as tc:
        with (
            tc.tile_pool(name="const", bufs=1) as constp,
            tc.tile_pool(name="psum", bufs=2, space="PSUM") as psump,
            tc.tile_pool(name="d2ep", bufs=4) as d2ep,
            tc.tile_pool(name="wp", bufs=4) as wp,
            tc.tile_pool(name="mfp", bufs=2) as mfp,
        ):
            qaug_sb = constp.tile([K, MPC], mybir.dt.bfloat16)
            nc.sync.dma_start(out=qaug_sb[:], in_=qaugT[:])
            daug_sb = constp.tile([K, N], mybir.dt.bfloat16)
            nc.sync.dma_start(out=daug_sb[:], in_=daugT[:])
            eps_t = constp.tile([128, 1], mybir.dt.float32)
            nc.vector.memset(eps_t[:], EPS)

            it = 0
            for b in range(BLOCKS):
                lhsT = qaug_sb[:, b * 128:(b + 1) * 128]
                for c in range(NCHUNK):
                    base = c * CHUNK
                    psum_t = psump.tile([128, CHUNK], mybir.dt.float32, tag="ps")
                    for s in range(SUB):
                        nc.tensor.matmul(
                            psum_t[:, s * 512:(s + 1) * 512],
                            lhsT=lhsT,
                            rhs=daug_sb[:, base + s * 512: base + (s + 1) * 512],
                            start=True, stop=True,
                        )
                    d2e = d2ep.tile([128, CHUNK], mybir.dt.float32, tag="d2e")
                    nc.scalar.activation(d2e[:], psum_t[:],
                                         mybir.ActivationFunctionType.Identity,
                                         bias=eps_t[:])
                    w_t = wp.tile([128, CHUNK], mybir.dt.bfloat16, tag="w")
                    if POOL_PATTERN[it % len(POOL_PATTERN)]:
                        mf = mfp.tile([128, CHUNK], mybir.dt.float32, tag="mf")
                        nc.gpsimd.tensor_single_scalar(
                            out=mf[:], in_=d2e[:], scalar=r2e,
                            op=mybir.AluOpType.is_le)
                        nc.gpsimd.tensor_tensor(
                            out=w_t[:], in0=mf[:], in1=d2e[:],
                            op=mybir.AluOpType.mult)
                    else:
                        nc.vector.scalar_tensor_tensor(
                            out=w_t[:], in0=psum_t[:], scalar=float(r2),
                            in1=d2e[:],
                            op0=mybir.AluOpType.is_le, op1=mybir.AluOpType.mult)
                    nc.sync.dma_start(
                        out=w_out[b * 128:(b + 1) * 128, base:base + CHUNK],
                        in_=w_t[:])
                    it += 1

    nc.finalize()
    return nc


def kernel(data, queries, radius):
    global LAST_RESULT
    data = np.asarray(data, np.float32)
    queries = np.asarray(queries, np.float32)
    r2 = np.float32(np.float32(radius) * np.float32(radius))

    lhsT_full, rhs_full = _build_aug(queries, data)

    key = float(r2)
    if key not in _nc_cache:
        _nc_cache[key] = _build_nc(float(r2))
    nc = _nc_cache[key]

    in_maps = [
        {
            "qaugT": np.ascontiguousarray(lhsT_full[:, c * MPC:(c + 1) * MPC]),
            "daugT": rhs_full,
        }
        for c in range(NCORES)
    ]
    res = run_bass_kernel_spmd(nc, in_maps, core_ids=list(range(NCORES)))
    LAST_RESULT = res

    w_bf = np.concatenate([r["w_out"] for r in res.results], axis=0)

    wu16 = w_bf.view(np.uint16)
    mask = wu16 != np.uint16(0)
    counts = np.count_nonzero(mask, axis=1)
    counts = np.minimum(counts, 255)
    row_splits = np.concatenate(
        [np.zeros(1, np.int64), np.cumsum(counts)]).astype(np.int32)
    # exact bf16 -> f32 via bit shift
    weights = (wu16.astype(np.uint32) << np.uint32(16)).view(np.float32)
    return row_splits, mask, weights


# revision 4
# speedup vs baseline: 3.8785x; 3.8785x over previous
"""Radius neighbor search (dense CSR encoding) on 8 TRN2 NeuronCores.

For M=12288 queries vs N=12288 data points in R^3 and radius r:
  d2[m,n]    = ||q_m||^2 + ||d_n||^2 - 2 q_m.d_n   (fp32-grade)
  mask[m,n]  = d2 <= r^2
  weights    = where(mask, max(d2,0), 0)           (fp32)
  row_splits = [0, cumsum(min(row_count, 255))]    (int32; the oracle's
               jnp.sum(mask, dtype=int32) saturates at 255 on this backend)

Sharding: queries row-parallel across 8 cores (1536 rows each), data
replicated; one SPMD Bass program.

Device computes only w = (d2 <= r2) * (d2 + eps) in bf16 (eps=1e-8 makes
every in-radius weight nonzero, so mask/counts derive from w != 0 on the
host). Per [128 x 2048] chunk:
  PE   : 4 matmuls, K=33 bf16 (3-way split of fp32 -> fp32-grade d2)
  ACT  : d2e = d2 + eps, PSUM -> SBUF f32
  DVE  : w = (d2_psum <= r2) * d2e   (scalar_tensor_tensor, ~60% of chunks)
  POOL : maskf = (d2e <= r2+eps); w = maskf * d2e  (~40% of chunks)
  DMA  : w chunk -> DRAM
"""
import numpy as np
import ml_dtypes

import concourse.bacc as bacc
import concourse.mybir as mybir
import concourse.tile as tile
from concourse.bass_utils import run_bass_kernel_spmd

BF16 = ml_dtypes.bfloat16

M = 12288
N = 12288
DIM = 3
NCORES = 8
MPC = M // NCORES   # 1536
K = 33

CHUNK = 2048
NCHUNK = N // CHUNK  # 6
BLOCKS = MPC // 128  # 12
SUB = CHUNK // 512   # 4
# eps exceeds the worst-case |d2| rounding error (~6e-7), so every
# in-radius weight (incl. exact-duplicate points, true d2=0) is strictly
# positive and mask == (w > 0) on the host.
EPS = 2e-6
# 4 of every 9 chunk-slots go to POOL (sign-multiply path)
POOL_PATTERN = [False, False, True, False, False, True, False, True, True]

LAST_RESULT = None
_nc_cache = {}


def _split3(x):
    x = np.asarray(x, np.float32)
    a = x.astype(BF16)
    r = x - a.astype(np.float32)
    b = r.astype(BF16)
    r2 = r - b.astype(np.float32)
    c = r2.astype(BF16)
    return a, b, c


def _build_aug(queries, data):
    """lhsT [K, M] bf16, rhs [K, N] bf16 with lhsT.T@rhs ~= d2 at fp32 grade."""
    q = np.asarray(queries, np.float32)
    d = np.asarray(data, np.float32)
    m, n = q.shape[0], d.shape[0]
    sq_q = np.sum(q * q, axis=1, dtype=np.float32)
    sq_d = np.sum(d * d, axis=1, dtype=np.float32)
    g = (-2.0 * d).astype(np.float32)

    qs = _split3(q)
    gs = _split3(g)
    ss = _split3(sq_q)
    ts_ = _split3(sq_d)

    ones_m = np.ones((m,), BF16)
    ones_n = np.ones((n,), BF16)
    lhs_rows, rhs_rows = [], []

    def add(lr, rr):
        lhs_rows.append(lr.astype(BF16))
        rhs_rows.append(rr.astype(BF16))

    add(ss[0], ones_n)
    add(ones_m, ts_[0])
    for k in range(DIM):
        add(qs[0][:, k], gs[0][:, k])
    add(ss[1], ones_n)
    add(ones_m, ts_[1])
    for (i, j) in [(0, 1), (1, 0)]:
        for k in range(DIM):
            add(qs[i][:, k], gs[j][:, k])
    add(ss[2], ones_n)
    add(ones_m, ts_[2])
    for (i, j) in [(1, 1), (0, 2), (2, 0), (1, 2), (2, 1), (2, 2)]:
        for k in range(DIM):
            add(qs[i][:, k], gs[j][:, k])

    lhsT = np.ascontiguousarray(np.stack(lhs_rows, axis=0))
    rhs = np.ascontiguousarray(np.stack(rhs_rows, axis=0))
    assert lhsT.shape == (K, m) and rhs.shape == (K, n)
    return lhsT, rhs


def _build_nc(r2: float):
    nc = bacc.Bacc("TRN2", target_bir_lowering=False, debug=False)
    qaugT = nc.dram_tensor("qaugT", [K, MPC], mybir.dt.bfloat16, kind="ExternalInput")
    daugT = nc.dram_tensor("daugT", [K, N], mybir.dt.bfloat16, kind="ExternalInput")
    w_out = nc.dram_tensor("w_out", [MPC, N], mybir.dt.bfloat16, kind="ExternalOutput")

    with tile.TileContext(nc) as tc:
        with (
            tc.tile_pool(name="const", bufs=1) as constp,
            tc.tile_pool(name="psum", bufs=2, space="PSUM") as psump,
            tc.tile_pool(name="d2ep", bufs=4) as d2ep,
            tc.tile_pool(name="wp", bufs=4) as wp,
            tc.tile_pool(name="mfp", bufs=2) as mfp,
        ):
            qaug_sb = constp.tile([K, MPC], mybir.dt.bfloat16)
            nc.sync.dma_start(out=qaug_sb[:], in_=qaugT[:])
            daug_sb = constp.tile([K, N], mybir.dt.bfloat16)
            nc.sync.dma_start(out=daug_sb[:], in_=daugT[:])
            eps_t = constp.tile([128, 1], mybir.dt.float32)
            nc.vector.memset(eps_t[:], EPS)
            r2_t = constp.tile([128, 1], mybir.dt.float32)
            nc.vector.memset(r2_t[:], float(r2))

            it = 0
            for b in range(BLOCKS):
                lhsT = qaug_sb[:, b * 128:(b + 1) * 128]
                for c in range(NCHUNK):
                    base = c * CHUNK
                    psum_t = psump.tile([128, CHUNK], mybir.dt.float32, tag="ps")
                    for s in range(SUB):
                        nc.tensor.matmul(
                            psum_t[:, s * 512:(s + 1) * 512],
                            lhsT=lhsT,
                            rhs=daug_sb[:, base + s * 512: base + (s + 1) * 512],
                            start=True, stop=True,
                        )
                    d2e = d2ep.tile([128, CHUNK], mybir.dt.float32, tag="d2e")
                    nc.scalar.activation(d2e[:], psum_t[:],
                                         mybir.ActivationFunctionType.Identity,
                                         bias=eps_t[:])
                    w_t = wp.tile([128, CHUNK], mybir.dt.bfloat16, tag="w")
                    if POOL_PATTERN[it % len(POOL_PATTERN)]:
                        sf = mfp.tile([128, CHUNK], mybir.dt.float32, tag="sf")
                        nc.scalar.activation(
                            sf[:], psum_t[:], mybir.ActivationFunctionType.Sign,
                            bias=r2_t[:], scale=-1.0)
                        nc.gpsimd.tensor_tensor(
                            out=w_t[:], in0=sf[:], in1=d2e[:],
                            op=mybir.AluOpType.mult)
                    else:
                        nc.vector.scalar_tensor_tensor(
                            out=w_t[:], in0=psum_t[:], scalar=float(r2),
                            in1=d2e[:],
                            op0=mybir.AluOpType.is_le, op1=mybir.AluOpType.mult)
                    nc.sync.dma_start(
                        out=w_out[b * 128:(b + 1) * 128, base:base + CHUNK],
                        in_=w_t[:])
                    it += 1

    nc.finalize()
    return nc


def kernel(data, queries, radius):
    global LAST_RESULT
    data = np.asarray(data, np.float32)
    queries = np.asarray(queries, np.float32)
    r2 = np.float32(np.float32(radius) * np.float32(radius))

    lhsT_full, rhs_full = _build_aug(queries, data)

    key = float(r2)
    if key not in _nc_cache:
        _nc_cache[key] = _build_nc(float(r2))
    nc = _nc_cache[key]

    in_maps = [
        {
            "qaugT": np.ascontiguousarray(lhsT_full[:, c * MPC:(c + 1) * MPC]),
            "daugT": rhs_full,
        }
        for c in range(NCORES)
    ]
    res = run_bass_kernel_spmd(nc, in_maps, core_ids=list(range(NCORES)))
    LAST_RESULT = res

    w_bf = np.concatenate([r["w_out"] for r in res.results], axis=0)

    wu16 = w_bf.view(np.uint16)
    # in-radius weights are strictly positive; POOL chunks encode
    # out-of-radius as negative (sign * d2e), host clamps them to 0
    mask = w_bf > np.float32(0)
    counts = np.count_nonzero(mask, axis=1)
    counts = np.minimum(counts, 255)
    row_splits = np.concatenate(
        [np.zeros(1, np.int64), np.cumsum(counts)]).astype(np.int32)
    # exact bf16 -> f32 via bit shift, then clamp negatives (out-of-radius)
    weights = (wu16.astype(np.uint32) << np.uint32(16)).view(np.float32)
    np.maximum(weights, 0.0, out=weights)
    return row_splits, mask, weights


# revision 5
# speedup vs baseline: 3.9446x; 1.0170x over previous
"""Radius neighbor search (dense CSR encoding) on 8 TRN2 NeuronCores.

For M=12288 queries vs N=12288 data points in R^3 and radius r:
  d2[m,n]    = ||q_m||^2 + ||d_n||^2 - 2 q_m.d_n   (fp32-grade)
  mask[m,n]  = d2 <= r^2
  weights    = where(mask, max(d2,0), 0)           (fp32)
  row_splits = [0, cumsum(min(row_count, 255))]    (int32; the oracle's
               jnp.sum(mask, dtype=int32) saturates at 255 on this backend)

Sharding: queries row-parallel across 8 cores (1536 rows each), data
replicated; one SPMD Bass program.

Device computes only w = (d2 <= r2) * (d2 + eps) in bf16 (eps=1e-8 makes
every in-radius weight nonzero, so mask/counts derive from w != 0 on the
host). Per [128 x 2048] chunk:
  PE   : 4 matmuls, K=33 bf16 (3-way split of fp32 -> fp32-grade d2)
  ACT  : d2e = d2 + eps, PSUM -> SBUF f32
  DVE  : w = (d2_psum <= r2) * d2e   (scalar_tensor_tensor, ~60% of chunks)
  POOL : maskf = (d2e <= r2+eps); w = maskf * d2e  (~40% of chunks)
  DMA  : w chunk -> DRAM
"""
import numpy as np
import ml_dtypes

import concourse.bacc as bacc
import concourse.mybir as mybir
import concourse.tile as tile
from concourse.bass_utils import run_bass_kernel_spmd

BF16 = ml_dtypes.bfloat16

M = 12288
N = 12288
DIM = 3
NCORES = 8
MPC = M // NCORES   # 1536
K = 33

CHUNK = 2048
NCHUNK = N // CHUNK  # 6
BLOCKS = MPC // 128  # 12
SUB = CHUNK // 512   # 4
# eps exceeds the worst-case |d2| rounding error (~6e-7), so every
# in-radius weight (incl. exact-duplicate points, true d2=0) is strictly
# positive and mask == (w > 0) on the host.
EPS = 2e-6
# 3 of every 8 chunk-slots go to POOL (sign-multiply path): balances
# ACT (72 copies + p signs) vs DVE ((72-p) selects) vs POOL (p multiplies)
POOL_PATTERN = [False, False, True, False, False, True, False, True]

LAST_RESULT = None
_nc_cache = {}


def _split3(x):
    x = np.asarray(x, np.float32)
    a = x.astype(BF16)
    r = x - a.astype(np.float32)
    b = r.astype(BF16)
    r2 = r - b.astype(np.float32)
    c = r2.astype(BF16)
    return a, b, c


def _build_aug(queries, data):
    """lhsT [K, M] bf16, rhs [K, N] bf16 with lhsT.T@rhs ~= d2 at fp32 grade."""
    q = np.asarray(queries, np.float32)
    d = np.asarray(data, np.float32)
    m, n = q.shape[0], d.shape[0]
    sq_q = np.sum(q * q, axis=1, dtype=np.float32)
    sq_d = np.sum(d * d, axis=1, dtype=np.float32)
    g = (-2.0 * d).astype(np.float32)

    qs = _split3(q)
    gs = _split3(g)
    ss = _split3(sq_q)
    ts_ = _split3(sq_d)

    ones_m = np.ones((m,), BF16)
    ones_n = np.ones((n,), BF16)
    lhs_rows, rhs_rows = [], []

    def add(lr, rr):
        lhs_rows.append(lr.astype(BF16))
        rhs_rows.append(rr.astype(BF16))

    add(ss[0], ones_n)
    add(ones_m, ts_[0])
    for k in range(DIM):
        add(qs[0][:, k], gs[0][:, k])
    add(ss[1], ones_n)
    add(ones_m, ts_[1])
    for (i, j) in [(0, 1), (1, 0)]:
        for k in range(DIM):
            add(qs[i][:, k], gs[j][:, k])
    add(ss[2], ones_n)
    add(ones_m, ts_[2])
    for (i, j) in [(1, 1), (0, 2), (2, 0), (1, 2), (2, 1), (2, 2)]:
        for k in range(DIM):
            add(qs[i][:, k], gs[j][:, k])

    lhsT = np.ascontiguousarray(np.stack(lhs_rows, axis=0))
    rhs = np.ascontiguousarray(np.stack(rhs_rows, axis=0))
    assert lhsT.shape == (K, m) and rhs.shape == (K, n)
    return lhsT, rhs


def _build_nc(r2: float):
    nc = bacc.Bacc("TRN2", target_bir_lowering=False, debug=False)
    qaugT = nc.dram_tensor("qaugT", [K, MPC], mybir.dt.bfloat16, kind="ExternalInput")
    daugT = nc.dram_tensor("daugT", [K, N], mybir.dt.bfloat16, kind="ExternalInput")
    w_out = nc.dram_tensor("w_out", [MPC, N], mybir.dt.bfloat16, kind="ExternalOutput")

    with tile.TileContext(nc) as tc:
        with (
            tc.tile_pool(name="const", bufs=1) as constp,
            tc.tile_pool(name="psum", bufs=2, space="PSUM") as psump,
            tc.tile_pool(name="d2ep", bufs=4) as d2ep,
            tc.tile_pool(name="wp", bufs=4) as wp,
            tc.tile_pool(name="mfp", bufs=2) as mfp,
        ):
            qaug_sb = constp.tile([K, MPC], mybir.dt.bfloat16)
            nc.sync.dma_start(out=qaug_sb[:], in_=qaugT[:])
            daug_sb = constp.tile([K, N], mybir.dt.bfloat16)
            nc.sync.dma_start(out=daug_sb[:], in_=daugT[:])
            eps_t = constp.tile([128, 1], mybir.dt.float32)
            nc.vector.memset(eps_t[:], EPS)
            r2_t = constp.tile([128, 1], mybir.dt.float32)
            nc.vector.memset(r2_t[:], float(r2))

            it = 0
            for b in range(BLOCKS):
                lhsT = qaug_sb[:, b * 128:(b + 1) * 128]
                for c in range(NCHUNK):
                    base = c * CHUNK
                    psum_t = psump.tile([128, CHUNK], mybir.dt.float32, tag="ps")
                    for s in range(SUB):
                        nc.tensor.matmul(
                            psum_t[:, s * 512:(s + 1) * 512],
                            lhsT=lhsT,
                            rhs=daug_sb[:, base + s * 512: base + (s + 1) * 512],
                            start=True, stop=True,
                        )
                    d2e = d2ep.tile([128, CHUNK], mybir.dt.float32, tag="d2e")
                    nc.scalar.activation(d2e[:], psum_t[:],
                                         mybir.ActivationFunctionType.Identity,
                                         bias=eps_t[:])
                    w_t = wp.tile([128, CHUNK], mybir.dt.bfloat16, tag="w")
                    if POOL_PATTERN[it % len(POOL_PATTERN)]:
                        sf = mfp.tile([128, CHUNK], mybir.dt.float32, tag="sf")
                        nc.scalar.activation(
                            sf[:], psum_t[:], mybir.ActivationFunctionType.Sign,
                            bias=r2_t[:], scale=-1.0)
                        nc.gpsimd.tensor_tensor(
                            out=w_t[:], in0=sf[:], in1=d2e[:],
                            op=mybir.AluOpType.mult)
                    else:
                        nc.vector.scalar_tensor_tensor(
                            out=w_t[:], in0=psum_t[:], scalar=float(r2),
                            in1=d2e[:],
                            op0=mybir.AluOpType.is_le, op1=mybir.AluOpType.mult)
                    nc.sync.dma_start(
                        out=w_out[b * 128:(b + 1) * 128, base:base + CHUNK],
                        in_=w_t[:])
                    it += 1

    nc.finalize()
    return nc


def kernel(data, queries, radius):
    global LAST_RESULT
    data = np.asarray(data, np.float32)
    queries = np.asarray(queries, np.float32)
    r2 = np.float32(np.float32(radius) * np.float32(radius))

    lhsT_full, rhs_full = _build_aug(queries, data)

    key = float(r2)
    if key not in _nc_cache:
        _nc_cache[key] = _build_nc(float(r2))
    nc = _nc_cache[key]

    in_maps = [
        {
            "qaugT": np.ascontiguousarray(lhsT_full[:, c * MPC:(c + 1) * MPC]),
            "daugT": rhs_full,
        }
        for c in range(NCORES)
    ]
    res = run_bass_kernel_spmd(nc, in_maps, core_ids=list(range(NCORES)))
    LAST_RESULT = res

    w_bf = np.concatenate([r["w_out"] for r in res.results], axis=0)

    wu16 = w_bf.view(np.uint16)
    # in-radius weights are strictly positive; POOL chunks encode
    # out-of-radius as negative (sign * d2e), host clamps them to 0
    mask = w_bf > np.float32(0)
    counts = np.count_nonzero(mask, axis=1)
    counts = np.minimum(counts, 255)
    row_splits = np.concatenate(
        [np.zeros(1, np.int64), np.cumsum(counts)]).astype(np.int32)
    # exact bf16 -> f32 via bit shift, then clamp negatives (out-of-radius)
    weights = (wu16.astype(np.uint32) << np.uint32(16)).view(np.float32)
    np.maximum(weights, 0.0, out=weights)
    return row_splits, mask, weights


# revision 6
# speedup vs baseline: 4.7345x; 1.2002x over previous
"""Radius neighbor search (dense CSR encoding) on 8 TRN2 NeuronCores.

For M=12288 queries vs N=12288 data points in R^3 and radius r:
  d2[m,n]    = ||q_m||^2 + ||d_n||^2 - 2 q_m.d_n   (fp32-grade)
  mask[m,n]  = d2 <= r^2
  weights    = where(mask, max(d2,0), 0)           (fp32)
  row_splits = [0, cumsum(min(row_count, 255))]    (int32; the oracle's
               jnp.sum(mask, dtype=int32) saturates at 255 on this backend)

Sharding: queries row-parallel across 8 cores (1536 rows each), data
replicated; one SPMD Bass program.

Device computes only w = (d2 <= r2) * (d2 + eps) in bf16 (eps=1e-8 makes
every in-radius weight nonzero, so mask/counts derive from w != 0 on the
host). Per [128 x 2048] chunk:
  PE   : 4 matmuls, K=33 bf16 (3-way split of fp32 -> fp32-grade d2)
  ACT  : d2e = d2 + eps, PSUM -> SBUF f32
  DVE  : w = (d2_psum <= r2) * d2e   (scalar_tensor_tensor, ~60% of chunks)
  POOL : maskf = (d2e <= r2+eps); w = maskf * d2e  (~40% of chunks)
  DMA  : w chunk -> DRAM
"""
import numpy as np
import ml_dtypes

import concourse.bacc as bacc
import concourse.mybir as mybir
import concourse.tile as tile
from concourse.bass_utils import run_bass_kernel_spmd

BF16 = ml_dtypes.bfloat16

M = 12288
N = 12288
DIM = 3
NCORES = 8
MPC = M // NCORES   # 1536
K = 33

CHUNK = 1024
NCHUNK = N // CHUNK  # 12
BLOCKS = MPC // 128  # 12
SUB = CHUNK // 512   # 2
# eps exceeds the worst-case |d2| rounding error (~6e-7), so every
# in-radius weight (incl. exact-duplicate points, true d2=0) is strictly
# positive and mask == (w > 0) on the host.
EPS = 2e-6
# 3 of every 8 chunk-slots go to POOL (sign-multiply path): balances
# ACT (72 copies + p signs) vs DVE ((72-p) selects) vs POOL (p multiplies)
POOL_PATTERN = [False, False, True, False, False, True, False, True]

LAST_RESULT = None
_nc_cache = {}


def _split3(x):
    x = np.asarray(x, np.float32)
    a = x.astype(BF16)
    r = x - a.astype(np.float32)
    b = r.astype(BF16)
    r2 = r - b.astype(np.float32)
    c = r2.astype(BF16)
    return a, b, c


def _build_aug(queries, data):
    """lhsT [K, M] bf16, rhs [K, N] bf16 with lhsT.T@rhs ~= d2 at fp32 grade."""
    q = np.asarray(queries, np.float32)
    d = np.asarray(data, np.float32)
    m, n = q.shape[0], d.shape[0]
    sq_q = np.sum(q * q, axis=1, dtype=np.float32)
    sq_d = np.sum(d * d, axis=1, dtype=np.float32)
    g = (-2.0 * d).astype(np.float32)

    qs = _split3(q)
    gs = _split3(g)
    ss = _split3(sq_q)
    ts_ = _split3(sq_d)

    ones_m = np.ones((m,), BF16)
    ones_n = np.ones((n,), BF16)
    lhs_rows, rhs_rows = [], []

    def add(lr, rr):
        lhs_rows.append(lr.astype(BF16))
        rhs_rows.append(rr.astype(BF16))

    add(ss[0], ones_n)
    add(ones_m, ts_[0])
    for k in range(DIM):
        add(qs[0][:, k], gs[0][:, k])
    add(ss[1], ones_n)
    add(ones_m, ts_[1])
    for (i, j) in [(0, 1), (1, 0)]:
        for k in range(DIM):
            add(qs[i][:, k], gs[j][:, k])
    add(ss[2], ones_n)
    add(ones_m, ts_[2])
    for (i, j) in [(1, 1), (0, 2), (2, 0), (1, 2), (2, 1), (2, 2)]:
        for k in range(DIM):
            add(qs[i][:, k], gs[j][:, k])

    lhsT = np.ascontiguousarray(np.stack(lhs_rows, axis=0))
    rhs = np.ascontiguousarray(np.stack(rhs_rows, axis=0))
    assert lhsT.shape == (K, m) and rhs.shape == (K, n)
    return lhsT, rhs


def _build_nc(r2: float):
    nc = bacc.Bacc("TRN2", target_bir_lowering=False, debug=False)
    qaugT = nc.dram_tensor("qaugT", [K, MPC], mybir.dt.bfloat16, kind="ExternalInput")
    daugT = nc.dram_tensor("daugT", [K, N], mybir.dt.bfloat16, kind="ExternalInput")
    w_out = nc.dram_tensor("w_out", [MPC, N], mybir.dt.bfloat16, kind="ExternalOutput")

    with tile.TileContext(nc) as tc:
        with (
            tc.tile_pool(name="const", bufs=1) as constp,
            tc.tile_pool(name="psum", bufs=4, space="PSUM") as psump,
            tc.tile_pool(name="d2ep", bufs=6) as d2ep,
            tc.tile_pool(name="wp", bufs=6) as wp,
            tc.tile_pool(name="mfp", bufs=4) as mfp,
        ):
            qaug_sb = constp.tile([K, MPC], mybir.dt.bfloat16)
            nc.sync.dma_start(out=qaug_sb[:], in_=qaugT[:])
            daug_sb = constp.tile([K, N], mybir.dt.bfloat16)
            nc.sync.dma_start(out=daug_sb[:], in_=daugT[:])
            eps_t = constp.tile([128, 1], mybir.dt.float32)
            nc.vector.memset(eps_t[:], EPS)
            r2_t = constp.tile([128, 1], mybir.dt.float32)
            nc.vector.memset(r2_t[:], float(r2))

            it = 0
            for b in range(BLOCKS):
                lhsT = qaug_sb[:, b * 128:(b + 1) * 128]
                for c in range(NCHUNK):
                    base = c * CHUNK
                    psum_t = psump.tile([128, CHUNK], mybir.dt.float32, tag="ps")
                    for s in range(SUB):
                        nc.tensor.matmul(
                            psum_t[:, s * 512:(s + 1) * 512],
                            lhsT=lhsT,
                            rhs=daug_sb[:, base + s * 512: base + (s + 1) * 512],
                            start=True, stop=True,
                        )
                    d2e = d2ep.tile([128, CHUNK], mybir.dt.float32, tag="d2e")
                    nc.scalar.activation(d2e[:], psum_t[:],
                                         mybir.ActivationFunctionType.Identity,
                                         bias=eps_t[:])
                    w_t = wp.tile([128, CHUNK], mybir.dt.bfloat16, tag="w")
                    if POOL_PATTERN[it % len(POOL_PATTERN)]:
                        sf = mfp.tile([128, CHUNK], mybir.dt.float32, tag="sf")
                        nc.scalar.activation(
                            sf[:], psum_t[:], mybir.ActivationFunctionType.Sign,
                            bias=r2_t[:], scale=-1.0)
                        nc.gpsimd.tensor_tensor(
                            out=w_t[:], in0=sf[:], in1=d2e[:],
                            op=mybir.AluOpType.mult)
                    else:
                        nc.vector.scalar_tensor_tensor(
                            out=w_t[:], in0=psum_t[:], scalar=float(r2),
                            in1=d2e[:],
                            op0=mybir.AluOpType.is_le, op1=mybir.AluOpType.mult)
                    nc.sync.dma_start(
                        out=w_out[b * 128:(b + 1) * 128, base:base + CHUNK],
                        in_=w_t[:])
                    it += 1

    nc.finalize()
    return nc


def kernel(data, queries, radius):
    global LAST_RESULT
    data = np.asarray(data, np.float32)
    queries = np.asarray(queries, np.float32)
    r2 = np.float32(np.float32(radius) * np.float32(radius))

    lhsT_full, rhs_full = _build_aug(queries, data)

    key = float(r2)
    if key not in _nc_cache:
        _nc_cache[key] = _build_nc(float(r2))
    nc = _nc_cache[key]

    in_maps = [
        {
            "qaugT": np.ascontiguousarray(lhsT_full[:, c * MPC:(c + 1) * MPC]),
            "daugT": rhs_full,
        }
        for c in range(NCORES)
    ]
    res = run_bass_kernel_spmd(nc, in_maps, core_ids=list(range(NCORES)))
    LAST_RESULT = res

    w_bf = np.concatenate([r["w_out"] for r in res.results], axis=0)

    wu16 = w_bf.view(np.uint16)
    # in-radius weights are strictly positive; POOL chunks encode
    # out-of-radius as negative (sign * d2e), host clamps them to 0
    mask = w_bf > np.float32(0)
    counts = np.count_nonzero(mask, axis=1)
    counts = np.minimum(counts, 255)
    row_splits = np.concatenate(
        [np.zeros(1, np.int64), np.cumsum(counts)]).astype(np.int32)
    # exact bf16 -> f32 via bit shift, then clamp negatives (out-of-radius)
    weights = (wu16.astype(np.uint32) << np.uint32(16)).view(np.float32)
    np.maximum(weights, 0.0, out=weights)
    return row_splits, mask, weights
